# revision 45
# baseline (speedup 1.0000x reference)
"""Trainium2 Bass kernel v3 for GQA attention (nn_Attention_74302934220843).

Tensor-parallel over heads (2 q-heads + 1 kv-head per core), AllToAll on the
attention output, token-sharded wo. v3 vs v2:
 - x / wq / wk / wv shipped and computed in bf16 (halves the dominant DMA
   stream; matmul cost keys on the moving operand which is 1 cyc/row for
   bf16 anyway).
 - causal mask folded into the scores as a PSUM pre-bias matmul (constant
   bf16 bias tile through an identity stationary), removing all DVE mask
   multiplies; exp then feeds PV/den directly.
 - rope entirely in bf16 (2x DVE mode) with bf16 reciprocal norms.
 - softmax denominator: per-group bf16 ci-combine add + in-place
   accumulate (no TensorScalarPtr), one ones-matmul per unit.
 - a2a split into 4 chunks (parity x q-half); odd q-blocks processed first
   so two collectives fire mid-attention; wo matmul groups for landed
   chunks are interleaved into the remaining attention stream (PSUM banks:
   scores 4 + out/den 3 + wo 1).
 - V transpose in bf16 (1.0 cyc/row, bf16 PSUM pass-through).
"""
import sys

for _p in ("/opt/trn_rl_repo", "/root/.axon_site/_ro/trn_rl_repo"):
    if _p not in sys.path:
        sys.path.insert(0, _p)

import numpy as np
import ml_dtypes
import concourse.bass as bass
import concourse.mybir as mybir
import concourse.tile as tile
from concourse import bacc
from concourse.bass_utils import run_bass_kernel_spmd

F32 = mybir.dt.float32
F32R = mybir.dt.float32r
BF16 = mybir.dt.bfloat16
AF = mybir.ActivationFunctionType
ALU = mybir.AluOpType

DIM = 2048
N_HEADS = 16
N_KV_HEADS = 4
HD = 128
EPS = 1e-6
BS = 2
NC_CORES = 8
HPC = N_HEADS // NC_CORES      # q heads per core = 2
ECH = DIM // 128               # e-chunks = 16
TBS = 512                      # token block size (phase 1)
QBS = 512                      # q block size (attention)
QH = QBS // 2                  # a2a chunk q-half
NEG = -30.0                    # causal-mask additive bias (pre-exp)


def build_program(seq=4096, no_collective=False, wo_start=4, wo_rate=2):
    T = BS * seq
    NTB = T // TBS                 # 16 token blocks
    QB = seq // QBS                # 8 q-blocks per batch
    TPC = T // NC_CORES            # 1024 tokens per core (output slice)

    nc = bacc.Bacc("TRN2", target_bir_lowering=False, debug=False,
                   num_devices=NC_CORES)

    xT = nc.dram_tensor("xT", [DIM, T], BF16, kind="ExternalInput").ap()
    wqT = nc.dram_tensor("wqT", [DIM, HPC * HD], BF16,
                           kind="ExternalInput").ap()
    wkT = nc.dram_tensor("wkT", [DIM, HD], BF16, kind="ExternalInput").ap()
    wvT = nc.dram_tensor("wvT", [DIM, HD], BF16, kind="ExternalInput").ap()
    woT = nc.dram_tensor("woT", [DIM, DIM], BF16, kind="ExternalInput").ap()
    ccd = nc.dram_tensor("ccd", [128, seq], BF16, kind="ExternalInput").ap()
    ssd = nc.dram_tensor("ssd", [128, seq], BF16, kind="ExternalInput").ap()
    biasd = nc.dram_tensor("biasd", [128, 2, QBS], BF16,
                           kind="ExternalInput").ap()
    biashd = nc.dram_tensor("biashd", [128, 2, QH], BF16,
                            kind="ExternalInput").ap()
    onesd = nc.dram_tensor("onesd", [128, 128], F32R,
                            kind="ExternalInput").ap()
    identbd = nc.dram_tensor("identbd", [128, 128], BF16,
                             kind="ExternalInput").ap()
    out = nc.dram_tensor("out", [TPC, DIM], F32, kind="ExternalOutput").ap()

    with tile.TileContext(nc) as tc:
        with (
            tc.tile_pool(name="singles", bufs=1) as singles,
            tc.tile_pool(name="dram", bufs=1, space="DRAM") as dram,
        ):
            # ---- kernel-lifetime resident SBUF ----
            ones_sb = singles.tile([128, 128], F32R)
            nc.sync.dma_start(out=ones_sb, in_=onesd)
            onesb_sb = singles.tile([128, 128], BF16)
            nc.scalar.copy(onesb_sb[:, :], ones_sb[:, :])
            identb_sb = singles.tile([128, 128], BF16)
            nc.sync.dma_start(out=identb_sb, in_=identbd)
            K_sb = singles.tile([128, T], BF16)          # normed+roped K (d major)
            V_sb = singles.tile([128, T // 128, HD], BF16)  # token-major V
            Q_sb = singles.tile([128, HPC, T], BF16)     # normed+roped Q (d major)
            ebq_sb = singles.tile([128, 1], F32)
            nc.vector.memset(ebq_sb, float(HD) * EPS)
            ebk_sb = singles.tile([128, 1], F32)
            nc.vector.memset(ebk_sb, EPS)

            # a2a chunks: 0 = odd q-blocks (2MB, fired mid-attention, fully
            # hidden); 1,2 = even q-block halves (1MB each, fired at the
            # end — the tail pipelines wo-odd || col1 -> wo1 || col2 -> wo2)
            a2a_in = [dram.tile([NC_CORES, HPC, HD, QBS], BF16,
                                name="a2a_in0")]
            a2a_out = [dram.tile([NC_CORES, HPC, HD, QBS], BF16,
                                 name="a2a_out0")]
            for c in (1, 2):
                a2a_in.append(dram.tile([NC_CORES, HPC, HD, QH], BF16,
                                        name=f"a2a_in{c}"))
                a2a_out.append(dram.tile([NC_CORES, HPC, HD, QH], BF16,
                                         name=f"a2a_out{c}"))

            # ================= Phase 1: projections =================
            with (
                tc.tile_pool(name="wsb", bufs=1) as wsb,
                tc.tile_pool(name="xt", bufs=4) as xtpool,
                tc.tile_pool(name="pk", bufs=2) as pkpool,
                tc.tile_pool(name="pstat", bufs=1) as pstat,
                tc.tile_pool(name="pstatr", bufs=2) as pstatr,
                tc.tile_pool(name="prope", bufs=2) as prope,
                tc.tile_pool(name="pjps", bufs=3, space="PSUM") as pjps,
                tc.tile_pool(name="statps", bufs=1, space="PSUM") as statps,
                tc.tile_pool(name="vtps", bufs=1, space="PSUM") as vtps,
            ):
                wq_sb = wsb.tile([128, ECH, HPC * HD], BF16)
                nc.scalar.dma_start(out=wq_sb,
                                    in_=wqT.rearrange("(ec p) m -> p ec m", p=128))
                wk_sb = wsb.tile([128, ECH, HD], BF16)
                nc.scalar.dma_start(out=wk_sb,
                                    in_=wkT.rearrange("(ec p) m -> p ec m", p=128))
                wv_sb = wsb.tile([128, ECH, HD], BF16)
                nc.scalar.dma_start(out=wv_sb,
                                    in_=wvT.rearrange("(ec p) m -> p ec m", p=128))
                cc_sb = wsb.tile([128, seq], BF16)
                nc.scalar.dma_start(out=cc_sb, in_=ccd)
                ss_sb = wsb.tile([128, seq], BF16)
                nc.scalar.dma_start(out=ss_sb, in_=ssd)

                xTr = xT.rearrange("(ec p) t -> p ec t", p=128)

                def emit_proj(tb, extras):
                    """Projection matmuls for block tb; pops one deferred
                    PE-op closure from `extras` every 2 e-chunks."""
                    t0 = tb * TBS
                    qq = pjps.tile([128, 2, TBS], F32, tag="pj",
                                   name=f"qq{tb}")
                    kv = pjps.tile([128, 2, TBS], F32, tag="pj",
                                   name=f"kv{tb}")
                    for half in range(4):
                        xh = xtpool.tile([128, ECH // 4, TBS], BF16,
                                         tag="xh")
                        nc.sync.dma_start(
                            out=xh,
                            in_=xTr[:, half * (ECH // 4):(half + 1) * (ECH // 4),
                                    t0:t0 + TBS])
                        for e8 in range(ECH // 4):
                            ec = half * (ECH // 4) + e8
                            st = ec == 0
                            sp = ec == ECH - 1
                            nc.tensor.matmul(qq[:, 0, :], wq_sb[:, ec, 0:128],
                                             xh[:, e8, :], start=st, stop=sp,
                                             skip_group_check=True)
                            nc.tensor.matmul(qq[:, 1, :], wq_sb[:, ec, 128:256],
                                             xh[:, e8, :], start=st, stop=sp,
                                             skip_group_check=True)
                            nc.tensor.matmul(kv[:, 0, :], wk_sb[:, ec, :],
                                             xh[:, e8, :], start=st, stop=sp,
                                             skip_group_check=True)
                            nc.tensor.matmul(kv[:, 1, :], wv_sb[:, ec, :],
                                             xh[:, e8, :], start=st, stop=sp,
                                             skip_group_check=True)
                            if ec % 2 == 1 and extras:
                                extras.pop(0)()
                    pkq = pkpool.tile([128, 2, TBS], BF16, tag="pkq")
                    nc.vector.tensor_copy(pkq[:, :, :], qq[:, :, :])
                    swq = pkpool.tile([128, 2, TBS], BF16, tag="swq")
                    nc.vector.tensor_copy(swq[0:64, :, :], pkq[64:128, :, :])
                    nc.vector.tensor_copy(swq[64:128, :, :], pkq[0:64, :, :])
                    pkv = pkpool.tile([128, 2, TBS], BF16, tag="pkv")
                    nc.vector.tensor_copy(pkv[:, :, :], kv[:, :, :])
                    swk = pkpool.tile([128, TBS], BF16, tag="swk")
                    nc.vector.tensor_copy(swk[0:64, :], pkv[64:128, 0, :])
                    nc.vector.tensor_copy(swk[64:128, :], pkv[0:64, 0, :])
                    return pkq, swq, pkv, swk

                def make_extras(tb, pkq, swq, pkv, swk):
                    """Deferred ops for a drained block: stats, rope, V
                    transpose."""
                    t0 = tb * TBS
                    s_off = t0 % seq
                    sq = pstat.tile([128, 3, TBS], F32R, tag="sq")
                    sv = pstat.tile([128, 3, TBS], F32, tag="sv")
                    rvv = pstatr.tile([128, 3, TBS], BF16, tag="rv")
                    ext = []

                    def sq_q():
                        nc.scalar.activation(sq[:, 0:2, :], pkq[:, :, :],
                                             AF.Square)
                        nc.scalar.activation(sq[:, 2, :], pkv[:, 0, :],
                                             AF.Square,
                                             scale=1.0 / float(np.sqrt(HD)))
                    ext.append(sq_q)

                    def stat(j):
                        def _f():
                            ssb = statps.tile([128, TBS], F32, tag="ss",
                                              name=f"ss{tb}_{j}")
                            nc.tensor.matmul(ssb[:, :], ones_sb[:, :],
                                             sq[:, j, :], start=True,
                                             stop=True, skip_group_check=True)
                            nc.scalar.activation(sv[:, j, :], ssb[:, :],
                                                 AF.Sqrt, bias=ebk_sb[:, :])
                        return _f
                    for j in range(3):
                        ext.append(stat(j))

                    def recip():
                        with nc.allow_low_precision(reason="bf16 rms recip"):
                            nc.vector.reciprocal(rvv[:, :, :], sv[:, :, :])
                    ext.append(recip)

                    def rope(src, srcsw, rvs, dst):
                        def _f():
                            cs = cc_sb[:, s_off:s_off + TBS]
                            sn = ss_sb[:, s_off:s_off + TBS]
                            pp = prope.tile([128, TBS], BF16, tag="pp")
                            nc.vector.tensor_mul(pp[:, :], src, cs)
                            qq_ = prope.tile([128, TBS], BF16, tag="qq_")
                            nc.vector.tensor_mul(qq_[:, :], srcsw, sn)
                            nc.vector.tensor_add(pp[:, :], pp[:, :],
                                                 qq_[:, :])
                            nc.vector.tensor_mul(dst, pp[:, :], rvs)
                        return _f
                    for h in range(HPC):
                        ext.append(rope(pkq[:, h, :], swq[:, h, :],
                                        rvv[:, h, :],
                                        Q_sb[:, h, t0:t0 + TBS]))
                    ext.append(rope(pkv[:, 0, :], swk[:, :], rvv[:, 2, :],
                                    K_sb[:, t0:t0 + TBS]))

                    vt = vtps.tile([128, 4, 128], BF16, tag="vt",
                                   name=f"vt{tb}")

                    def vtr(cch):
                        def _f():
                            nc.tensor.transpose(
                                vt[:, cch, :],
                                pkv[:, 1, cch * 128:(cch + 1) * 128],
                                identb_sb[:, :])
                        return _f
                    for cch in range(4):
                        ext.append(vtr(cch))

                    def vcopy():
                        nc.scalar.copy(V_sb[:, tb * 4:(tb + 1) * 4, :],
                                       vt[:, :, :])
                    ext.append(vcopy)
                    return ext

                extras = []
                for tb in range(NTB):
                    drains = emit_proj(tb, extras)
                    extras = extras + make_extras(tb, *drains)
                for e in extras:
                    e()

            def emit_collective(c):
                if no_collective:
                    nc.sync.dma_start(out=a2a_out[c], in_=a2a_in[c])
                else:
                    nc.gpsimd.collective_compute(
                        "AllToAll", ALU.bypass,
                        replica_groups=[list(range(NC_CORES))],
                        ins=[a2a_in[c].opt()], outs=[a2a_out[c].opt()],
                    )

            # ============ Phase 2+3: attention, collectives, wo ============
            with (
                tc.tile_pool(name="battn", bufs=1) as battn,
                tc.tile_pool(name="apt", bufs=6) as aptpool,
                tc.tile_pool(name="aden", bufs=2) as adenpool,
                tc.tile_pool(name="amisc", bufs=4) as amisc,
                tc.tile_pool(name="wao", bufs=1) as waopool,
                tc.tile_pool(name="wwt", bufs=64) as wwtpool,
                tc.tile_pool(name="wdr", bufs=2) as wdrpool,
                tc.tile_pool(name="sps", bufs=2, space="PSUM") as spsps,
                tc.tile_pool(name="outps", bufs=3, space="PSUM") as outps,
                tc.tile_pool(name="wops", bufs=1, space="PSUM") as wops,
            ):
                bias_sb = battn.tile([128, 2, QBS], BF16)
                nc.scalar.dma_start(out=bias_sb, in_=biasd)
                biash_sb = battn.tile([128, 2, QH], BF16)
                nc.scalar.dma_start(out=biash_sb, in_=biashd)

                # wo weight tiles, loaded once, fully resident (64KB/part)
                wt_tiles = {}
                for eb in range(DIM // 512):
                    for hc in range(ECH):
                        wt = wwtpool.tile([128, 512], BF16, tag="wt",
                                          name=f"wt{eb}_{hc}")
                        nc.sync.dma_start(
                            out=wt,
                            in_=woT[hc * 128:(hc + 1) * 128,
                                    eb * 512:(eb + 1) * 512])
                        wt_tiles[(eb, hc)] = wt

                ao_sbs = {}

                def load_aosb(c):
                    w = QBS if c == 0 else QH
                    ao_sb = waopool.tile([128, ECH, w], BF16, bufs=1,
                                         tag=f"aosb{c}", name=f"aosb{c}")
                    aor = a2a_out[c].rearrange("r h p t -> p (r h) t")
                    for hc in range(ECH):
                        nc.gpsimd.dma_start(out=ao_sb[:, hc, :],
                                            in_=aor[:, hc, :])
                    ao_sbs[c] = ao_sb

                # wo work: one closure per (chunk, eb, tt) = 16 matmuls;
                # chunk 0 has tt 0..3 (512 q), chunks 1,2 have tt 0..1
                def wo_group(c, eb, tt):
                    def _f():
                        ao_sb = ao_sbs[c]
                        ops = wops.tile([128, 512], F32, tag="wps",
                                        name=f"wps{c}_{eb}_{tt}")
                        for hc in range(ECH):
                            nc.tensor.matmul(
                                ops[:, :],
                                ao_sb[:, hc, tt * 128:(tt + 1) * 128],
                                wt_tiles[(eb, hc)],
                                start=(hc == 0), stop=(hc == ECH - 1),
                                skip_group_check=True)
                        od = wdrpool.tile([128, 512], F32, tag="od")
                        if (eb + tt) % 2 == 0:
                            nc.scalar.copy(od[:, :], ops[:, :])
                        else:
                            nc.vector.tensor_copy(od[:, :], ops[:, :])
                        row = (QBS if c == 0 else (c - 1) * QH) + tt * 128
                        nc.sync.dma_start(
                            out=out[row:row + 128,
                                    eb * 512:(eb + 1) * 512],
                            in_=od)
                    return _f

                wo_pending = []

                # software-pipelined PV: deferred one group behind scores
                pending = []

                def flush_pv():
                    while pending:
                        pt, o_ps, b_, g_, ng_ = pending.pop(0)
                        last = g_ == ng_ - 1
                        qw = QH if last else QBS
                        q0 = QBS - qw
                        for ci in range(2):
                            kcol = b_ * seq + g_ * 256 + ci * 128
                            nc.tensor.matmul(
                                o_ps[:, q0:QBS],
                                V_sb[:, kcol // 128, :],
                                pt[:, ci, 0:qw],
                                start=(g_ == 0 and ci == 0),
                                stop=(last and ci == 1),
                                skip_group_check=True)

                finish = []

                def flush_finish():
                    while finish:
                        finish.pop(0)()

                # odd q-blocks first (their a2a chunk fires mid-attention and
                # is fully hidden by the even phase); within each parity, big
                # and small q-blocks alternate so small units' serial finish
                # chains hide inside big units' PE work.
                def mix_qbs(par):
                    qbs = list(range(QB - 1 - (1 - par), -1, -2))
                    mix = []
                    while qbs:
                        mix.append(qbs.pop(0))       # biggest
                        if qbs:
                            mix.append(qbs.pop())    # smallest
                    return mix

                unit_order = [(b, qb) for par in (1, 0)
                              for qb in mix_qbs(par)
                              for b in range(BS)]
                n_odd = len(unit_order) // 2
                last_ao = [None]
                for ui, (b, qb) in enumerate(unit_order):
                    tok0 = b * seq + qb * QBS
                    ng = 2 * (qb + 1)
                    for h in range(HPC):
                        o_ps = outps.tile([128, QBS], F32, tag="ops")
                        den_sb = adenpool.tile([128, QBS], BF16, tag="den")
                        for g in range(ng):
                            last = g == ng - 1
                            qw = QH if last else QBS
                            q0 = QBS - qw
                            sps = spsps.tile([128, 2, QBS], F32, tag="sps")
                            if g == ng - 2:
                                for ci in range(2):
                                    nc.tensor.matmul(
                                        sps[:, ci, :], identb_sb[:, :],
                                        bias_sb[:, ci, :], start=True,
                                        stop=False, skip_group_check=True)
                            elif last:
                                for ci in range(2):
                                    nc.tensor.matmul(
                                        sps[:, ci, 0:qw], identb_sb[:, :],
                                        biash_sb[:, ci, :], start=True,
                                        stop=False, skip_group_check=True)
                            for ci in range(2):
                                kcol = b * seq + g * 256 + ci * 128
                                nc.tensor.matmul(
                                    sps[:, ci, 0:qw],
                                    K_sb[:, kcol:kcol + 128],
                                    Q_sb[:, h, tok0 + q0:tok0 + QBS],
                                    start=(g < ng - 2), stop=True,
                                    skip_group_check=True)
                            flush_pv()
                            flush_finish()
                            pt = aptpool.tile([128, 2, QBS], BF16, tag="pt")
                            nc.scalar.activation(pt[:, :, 0:qw],
                                                 sps[:, :, 0:qw], AF.Exp)
                            pending.append((pt, o_ps, b, g, ng))
                            if g == 0:
                                nc.vector.tensor_add(den_sb[:, :],
                                                     pt[:, 0, :],
                                                     pt[:, 1, :])
                            else:
                                tmp = amisc.tile([128, QBS], BF16,
                                                 tag="dtmp")
                                nc.vector.tensor_add(
                                    tmp[:, 0:qw],
                                    pt[:, 0, 0:qw],
                                    pt[:, 1, 0:qw])
                                nc.vector.tensor_add(
                                    den_sb[:, q0:QBS],
                                    tmp[:, 0:qw],
                                    den_sb[:, q0:QBS])

                        def unit_finish(o_ps=o_ps, den_sb=den_sb,
                                        tok0=tok0, h=h, qb=qb):
                            den_ps = outps.tile([128, QBS], F32, tag="ops",
                                                name=f"dps{tok0}_{h}")
                            nc.tensor.matmul(den_ps[:, :], onesb_sb[:, :],
                                             den_sb[:, :], start=True,
                                             stop=True, skip_group_check=True)
                            rv = amisc.tile([128, QBS], F32, tag="arv")
                            nc.vector.reciprocal(rv[:, :], den_ps[:, :])
                            ao = amisc.tile([128, QBS], BF16, tag="ao")
                            nc.vector.tensor_mul(ao[:, :], o_ps[:, :],
                                                 rv[:, :])
                            last_ao[0] = ao
                            d = tok0 // TPC
                            if qb % 2 == 1:
                                nc.sync.dma_start(
                                    out=a2a_in[0][d, h, :, :], in_=ao[:, :])
                            else:
                                for j in (1, 2):
                                    nc.sync.dma_start(
                                        out=a2a_in[j][d, h, :, :],
                                        in_=ao[:, (j - 1) * QH:j * QH])

                        finish.append(unit_finish)
                    if ui == n_odd - 1:
                        flush_pv()
                        flush_finish()
                        emit_collective(0)
                    if ui == n_odd + wo_start:
                        # Pre-write garbage (a real data dep on the current
                        # attention unit) into one column of every (hc, tt)
                        # region of ao_sb0 halves, THEN emit the loads that
                        # overwrite it. The tile scheduler models collectives
                        # as ~instant; without this extra writer it would
                        # place chunk-0 wo matmuls at a stream position
                        # reached long before the collective lands at
                        # runtime, freezing the in-order PE pipe. With it,
                        # the scheduler's readiness estimate for wo (~this
                        # unit's finish) matches runtime (collective landed).
                        # The q-half tt 2,3 is gated on the LAST even unit so
                        # its 8 wo groups remain to cover the collective-1
                        # window in the tail.
                        ao_sb = waopool.tile([128, ECH, QBS], BF16, bufs=1,
                                             tag="aosb0", name="aosb0")
                        ao_sbs[0] = ao_sb
                        for tt in range(4):
                            nc.vector.tensor_copy(
                                ao_sb[:, :, tt * 128],
                                last_ao[0][:, 0:ECH])
                        aor = a2a_out[0].rearrange("r h p t -> p (r h) t")
                        for hc in range(ECH):
                            nc.gpsimd.dma_start(out=ao_sb[:, hc, :],
                                                in_=aor[:, hc, :])
                flush_pv()
                flush_finish()
                emit_collective(1)
                emit_collective(2)
                load_aosb(1)
                load_aosb(2)
                # wo for the odd chunk runs while collectives 1+2 transfer
                for eb in range(DIM // 512):
                    for tt in range(4):
                        wo_group(0, eb, tt)()
                for eb in range(DIM // 512):
                    for tt in range(2):
                        wo_group(1, eb, tt)()
                for eb in range(DIM // 512):
                    for tt in range(2):
                        wo_group(2, eb, tt)()
    nc.compile()
    return nc


# ---------------- host-side prep / run ----------------

_PROG_CACHE = {}


def _get_program(seq):
    if seq not in _PROG_CACHE:
        _PROG_CACHE[seq] = build_program(seq)
    return _PROG_CACHE[seq]


def _rot_perm():
    return np.concatenate([np.arange(0, HD, 2), np.arange(1, HD, 2)])


def make_inputs(x, freqs_cis, wq, wk, wv, wo, q_norm_w, k_norm_w):
    bs, seq, _ = x.shape
    T = bs * seq
    perm = _rot_perm()

    xT = np.ascontiguousarray(
        x.reshape(T, DIM).T.astype(ml_dtypes.bfloat16))
    woT = np.ascontiguousarray(wo.T.astype(ml_dtypes.bfloat16))
    cos = freqs_cis[:, :, 0].T.astype(np.float32)   # [64, seq]
    sin = freqs_cis[:, :, 1].T.astype(np.float32)
    ccd = np.ascontiguousarray(
        np.concatenate([cos, cos], axis=0).astype(ml_dtypes.bfloat16))
    ssd = np.ascontiguousarray(
        np.concatenate([-sin, sin], axis=0).astype(ml_dtypes.bfloat16))

    # additive causal-mask bias tiles (0 allowed / NEG masked)
    # full tile: second-to-last 256-key group vs all 512 q
    # half tile: last 256-key group vs q in [256, 512) only
    masks = np.zeros((2, 128, 2, QBS), dtype=np.float32)
    for s in range(2):
        for c in range(2):
            k_rel = s * 256 + c * 128 + np.arange(128)[:, None]
            masks[s, :, c, :] = (k_rel <= np.arange(QBS)[None, :])
    # unmasked -> 0, masked -> NEG
    biasd = np.ascontiguousarray(
        ((masks[0] - 1.0) * (-NEG)).astype(ml_dtypes.bfloat16))
    biashd = np.ascontiguousarray(
        ((masks[1, :, :, QH:] - 1.0) * (-NEG)).astype(ml_dtypes.bfloat16))

    onesd = np.ones((128, 128), dtype=np.float32)
    identbd = np.eye(128, dtype=ml_dtypes.bfloat16)

    in_maps = []
    for c in range(NC_CORES):
        g = c // 2
        wq_rows = wq[c * HPC * HD:(c + 1) * HPC * HD].reshape(HPC, HD, DIM)
        wq_rows = wq_rows[:, perm, :].reshape(HPC * HD, DIM)
        wk_rows = wk[g * HD:(g + 1) * HD][perm]
        wv_rows = wv[g * HD:(g + 1) * HD]
        in_maps.append({
            "xT": xT,
            "wqT": np.ascontiguousarray(
                wq_rows.T.astype(ml_dtypes.bfloat16)),
            "wkT": np.ascontiguousarray(
                wk_rows.T.astype(ml_dtypes.bfloat16)),
            "wvT": np.ascontiguousarray(
                wv_rows.T.astype(ml_dtypes.bfloat16)),
            "woT": woT,
            "ccd": ccd,
            "ssd": ssd,
            "biasd": biasd,
            "biashd": biashd,
            "onesd": onesd,
            "identbd": identbd,
        })
    return in_maps


def run(x, freqs_cis, wq, wk, wv, wo, q_norm_w, k_norm_w, trace=False):
    bs, seq, _ = x.shape
    nc = _get_program(seq)
    in_maps = make_inputs(x, freqs_cis, wq, wk, wv, wo, q_norm_w, k_norm_w)
    res = None
    for attempt in range(3):
        try:
            res = run_bass_kernel_spmd(nc, in_maps, list(range(NC_CORES)),
                                       trace=trace)
            break
        except Exception:
            if attempt == 2:
                raise
    shards = [res.results[c]["out"] for c in range(NC_CORES)]
    full = np.concatenate(shards, axis=0).reshape(bs, seq, DIM)
    return full, res


def kernel(x, freqs_cis, wq, wk, wv, wo, q_norm_w, k_norm_w):
    out, _ = run(np.asarray(x, np.float32), np.asarray(freqs_cis, np.float32),
                 np.asarray(wq, np.float32), np.asarray(wk, np.float32),
                 np.asarray(wv, np.float32), np.asarray(wo, np.float32),
                 np.asarray(q_norm_w, np.float32), np.asarray(k_norm_w, np.float32))
    return out


# revision 54
# speedup vs baseline: 1.2169x; 1.2169x over previous
"""Trainium2 Bass kernel v3 for GQA attention (nn_Attention_74302934220843).

Tensor-parallel over heads (2 q-heads + 1 kv-head per core), AllToAll on the
attention output, token-sharded wo. v3 vs v2:
 - x / wq / wk / wv shipped and computed in bf16 (halves the dominant x DMA
   stream; matmul cost keys on the moving operand which is 1 cyc/row for
   bf16 anyway). rope/rms-norm entirely in bf16 (2x DVE mode).
 - softmax denominator: per-group bf16 ci-combine add + plain in-place
   accumulate adds (the TensorScalarPtr accumulate was ~3x slower), one
   ones-matmul per unit into PSUM.
 - a2a split into 3 chunks: odd q-blocks (2MB) fired mid-attention and
   fully hidden; the even q-block halves (1MB each) fired at the end so
   the tail pipelines wo-odd || col1 -> wo1 || col2 -> wo2.
 - wo weight tiles loaded once (not per chunk), fully SBUF-resident.
 - chunk-0 wo matmuls interleave into the Act-bound even-parity attention
   stream. The tile scheduler models collectives as ~instant, so ao_sb
   gets a garbage pre-write with a real data dep on a mid-phase attention
   unit before the loads: the scheduler's readiness estimate for wo then
   matches runtime and the in-order PE stream never freezes on an
   un-landed collective (that freeze cost ~43us).
 - V transpose in bf16 (1.0 cyc/row, bf16 PSUM pass-through).
 - PSUM banks: scores 4 + out/den(shared pool) 3 + wo 1.
"""
import sys

for _p in ("/opt/trn_rl_repo", "/root/.axon_site/_ro/trn_rl_repo"):
    if _p not in sys.path:
        sys.path.insert(0, _p)

import numpy as np
import ml_dtypes
import concourse.bass as bass
import concourse.mybir as mybir
import concourse.tile as tile
from concourse import bacc
from concourse.bass_utils import run_bass_kernel_spmd

F32 = mybir.dt.float32
F32R = mybir.dt.float32r
BF16 = mybir.dt.bfloat16
AF = mybir.ActivationFunctionType
ALU = mybir.AluOpType

DIM = 2048
N_HEADS = 16
N_KV_HEADS = 4
HD = 128
EPS = 1e-6
BS = 2
NC_CORES = 8
HPC = N_HEADS // NC_CORES      # q heads per core = 2
ECH = DIM // 128               # e-chunks = 16
TBS = 512                      # token block size (phase 1)
QBS = 512                      # q block size (attention)
QH = QBS // 2                  # a2a chunk q-half
NEG = -30.0                    # causal-mask additive bias (pre-exp)


def build_program(seq=4096, no_collective=False, wo_start=4, wo_rate=2):
    T = BS * seq
    NTB = T // TBS                 # 16 token blocks
    QB = seq // QBS                # 8 q-blocks per batch
    TPC = T // NC_CORES            # 1024 tokens per core (output slice)

    nc = bacc.Bacc("TRN2", target_bir_lowering=False, debug=False,
                   num_devices=NC_CORES)

    xT = nc.dram_tensor("xT", [DIM, T], BF16, kind="ExternalInput").ap()
    wqT = nc.dram_tensor("wqT", [DIM, HPC * HD], BF16,
                           kind="ExternalInput").ap()
    wkT = nc.dram_tensor("wkT", [DIM, HD], BF16, kind="ExternalInput").ap()
    wvT = nc.dram_tensor("wvT", [DIM, HD], BF16, kind="ExternalInput").ap()
    woT = nc.dram_tensor("woT", [DIM, DIM], BF16, kind="ExternalInput").ap()
    ccd = nc.dram_tensor("ccd", [128, seq], BF16, kind="ExternalInput").ap()
    ssd = nc.dram_tensor("ssd", [128, seq], BF16, kind="ExternalInput").ap()
    biasd = nc.dram_tensor("biasd", [128, 2, QBS], BF16,
                           kind="ExternalInput").ap()
    biashd = nc.dram_tensor("biashd", [128, 2, QH], BF16,
                            kind="ExternalInput").ap()
    onesd = nc.dram_tensor("onesd", [128, 128], F32R,
                            kind="ExternalInput").ap()
    identbd = nc.dram_tensor("identbd", [128, 128], BF16,
                             kind="ExternalInput").ap()
    out = nc.dram_tensor("out", [TPC, DIM], F32, kind="ExternalOutput").ap()

    with tile.TileContext(nc) as tc:
        with (
            tc.tile_pool(name="singles", bufs=1) as singles,
            tc.tile_pool(name="dram", bufs=1, space="DRAM") as dram,
        ):
            # ---- kernel-lifetime resident SBUF ----
            ones_sb = singles.tile([128, 128], F32R)
            nc.sync.dma_start(out=ones_sb, in_=onesd)
            onesb_sb = singles.tile([128, 128], BF16)
            nc.scalar.copy(onesb_sb[:, :], ones_sb[:, :])
            identb_sb = singles.tile([128, 128], BF16)
            nc.sync.dma_start(out=identb_sb, in_=identbd)
            K_sb = singles.tile([128, T], BF16)          # normed+roped K (d major)
            V_sb = singles.tile([128, T // 128, HD], BF16)  # token-major V
            Q_sb = singles.tile([128, HPC, T], BF16)     # normed+roped Q (d major)
            ebq_sb = singles.tile([128, 1], F32)
            nc.vector.memset(ebq_sb, float(HD) * EPS)
            ebk_sb = singles.tile([128, 1], F32)
            nc.vector.memset(ebk_sb, EPS)

            # a2a chunks: 0 = odd q-blocks (2MB, fired mid-attention, fully
            # hidden); 1,2 = even q-block halves (1MB each, fired at the
            # end — the tail pipelines wo-odd || col1 -> wo1 || col2 -> wo2)
            a2a_in = [dram.tile([NC_CORES, HPC, HD, QBS], BF16,
                                name="a2a_in0")]
            a2a_out = [dram.tile([NC_CORES, HPC, HD, QBS], BF16,
                                 name="a2a_out0")]
            for c in (1, 2):
                a2a_in.append(dram.tile([NC_CORES, HPC, HD, QH], BF16,
                                        name=f"a2a_in{c}"))
                a2a_out.append(dram.tile([NC_CORES, HPC, HD, QH], BF16,
                                         name=f"a2a_out{c}"))

            # ================= Phase 1: projections =================
            with (
                tc.tile_pool(name="wsb", bufs=1) as wsb,
                tc.tile_pool(name="xt", bufs=3) as xtpool,
                tc.tile_pool(name="pk", bufs=2) as pkpool,
                tc.tile_pool(name="pstat", bufs=1) as pstat,
                tc.tile_pool(name="pstatr", bufs=2) as pstatr,
                tc.tile_pool(name="prope", bufs=2) as prope,
                tc.tile_pool(name="pjps", bufs=3, space="PSUM") as pjps,
                tc.tile_pool(name="statps", bufs=1, space="PSUM") as statps,
                tc.tile_pool(name="vtps", bufs=1, space="PSUM") as vtps,
            ):
                wq_sb = wsb.tile([128, ECH, HPC * HD], BF16)
                nc.scalar.dma_start(out=wq_sb,
                                    in_=wqT.rearrange("(ec p) m -> p ec m", p=128))
                wk_sb = wsb.tile([128, ECH, HD], BF16)
                nc.scalar.dma_start(out=wk_sb,
                                    in_=wkT.rearrange("(ec p) m -> p ec m", p=128))
                wv_sb = wsb.tile([128, ECH, HD], BF16)
                nc.scalar.dma_start(out=wv_sb,
                                    in_=wvT.rearrange("(ec p) m -> p ec m", p=128))
                cc_sb = wsb.tile([128, seq], BF16)
                nc.scalar.dma_start(out=cc_sb, in_=ccd)
                ss_sb = wsb.tile([128, seq], BF16)
                nc.scalar.dma_start(out=ss_sb, in_=ssd)

                xTr = xT.rearrange("(ec p) t -> p ec t", p=128)

                def emit_proj(tb, extras):
                    """Projection matmuls for block tb; pops one deferred
                    PE-op closure from `extras` every 2 e-chunks."""
                    t0 = tb * TBS
                    qq = pjps.tile([128, 2, TBS], F32, tag="pj",
                                   name=f"qq{tb}")
                    kv = pjps.tile([128, 2, TBS], F32, tag="pj",
                                   name=f"kv{tb}")
                    for half in range(4):
                        xh = xtpool.tile([128, ECH // 4, TBS], BF16,
                                         tag="xh")
                        nc.sync.dma_start(
                            out=xh,
                            in_=xTr[:, half * (ECH // 4):(half + 1) * (ECH // 4),
                                    t0:t0 + TBS])
                        for e8 in range(ECH // 4):
                            ec = half * (ECH // 4) + e8
                            st = ec == 0
                            sp = ec == ECH - 1
                            nc.tensor.matmul(qq[:, 0, :], wq_sb[:, ec, 0:128],
                                             xh[:, e8, :], start=st, stop=sp,
                                             skip_group_check=True)
                            nc.tensor.matmul(qq[:, 1, :], wq_sb[:, ec, 128:256],
                                             xh[:, e8, :], start=st, stop=sp,
                                             skip_group_check=True)
                            nc.tensor.matmul(kv[:, 0, :], wk_sb[:, ec, :],
                                             xh[:, e8, :], start=st, stop=sp,
                                             skip_group_check=True)
                            nc.tensor.matmul(kv[:, 1, :], wv_sb[:, ec, :],
                                             xh[:, e8, :], start=st, stop=sp,
                                             skip_group_check=True)
                            if ec % 2 == 1 and extras:
                                extras.pop(0)()
                    pkq = pkpool.tile([128, 2, TBS], BF16, tag="pkq")
                    nc.vector.tensor_copy(pkq[:, :, :], qq[:, :, :])
                    swq = pkpool.tile([128, 2, TBS], BF16, tag="swq")
                    nc.vector.tensor_copy(swq[0:64, :, :], pkq[64:128, :, :])
                    nc.vector.tensor_copy(swq[64:128, :, :], pkq[0:64, :, :])
                    pkv = pkpool.tile([128, 2, TBS], BF16, tag="pkv")
                    nc.vector.tensor_copy(pkv[:, :, :], kv[:, :, :])
                    swk = pkpool.tile([128, TBS], BF16, tag="swk")
                    nc.vector.tensor_copy(swk[0:64, :], pkv[64:128, 0, :])
                    nc.vector.tensor_copy(swk[64:128, :], pkv[0:64, 0, :])
                    return pkq, swq, pkv, swk

                def make_extras(tb, pkq, swq, pkv, swk):
                    """Deferred ops for a drained block: stats, rope, V
                    transpose."""
                    t0 = tb * TBS
                    s_off = t0 % seq
                    sq = pstat.tile([128, 3, TBS], F32R, tag="sq")
                    sv = pstat.tile([128, 3, TBS], F32, tag="sv")
                    rvv = pstatr.tile([128, 3, TBS], BF16, tag="rv")
                    ext = []

                    def sq_q():
                        nc.scalar.activation(sq[:, 0:2, :], pkq[:, :, :],
                                             AF.Square)
                        nc.scalar.activation(sq[:, 2, :], pkv[:, 0, :],
                                             AF.Square,
                                             scale=1.0 / float(np.sqrt(HD)))
                    ext.append(sq_q)

                    def stat(j):
                        def _f():
                            ssb = statps.tile([128, TBS], F32, tag="ss",
                                              name=f"ss{tb}_{j}")
                            nc.tensor.matmul(ssb[:, :], ones_sb[:, :],
                                             sq[:, j, :], start=True,
                                             stop=True, skip_group_check=True)
                            nc.scalar.activation(sv[:, j, :], ssb[:, :],
                                                 AF.Sqrt, bias=ebk_sb[:, :])
                        return _f
                    for j in range(3):
                        ext.append(stat(j))

                    def recip():
                        with nc.allow_low_precision(reason="bf16 rms recip"):
                            nc.vector.reciprocal(rvv[:, :, :], sv[:, :, :])
                    ext.append(recip)

                    def rope(src, srcsw, rvs, dst):
                        def _f():
                            cs = cc_sb[:, s_off:s_off + TBS]
                            sn = ss_sb[:, s_off:s_off + TBS]
                            pp = prope.tile([128, TBS], BF16, tag="pp")
                            nc.vector.tensor_mul(pp[:, :], src, cs)
                            qq_ = prope.tile([128, TBS], BF16, tag="qq_")
                            nc.vector.tensor_mul(qq_[:, :], srcsw, sn)
                            nc.vector.tensor_add(pp[:, :], pp[:, :],
                                                 qq_[:, :])
                            nc.vector.tensor_mul(dst, pp[:, :], rvs)
                        return _f
                    for h in range(HPC):
                        ext.append(rope(pkq[:, h, :], swq[:, h, :],
                                        rvv[:, h, :],
                                        Q_sb[:, h, t0:t0 + TBS]))
                    ext.append(rope(pkv[:, 0, :], swk[:, :], rvv[:, 2, :],
                                    K_sb[:, t0:t0 + TBS]))

                    vt = vtps.tile([128, 4, 128], BF16, tag="vt",
                                   name=f"vt{tb}")

                    def vtr(cch):
                        def _f():
                            nc.tensor.transpose(
                                vt[:, cch, :],
                                pkv[:, 1, cch * 128:(cch + 1) * 128],
                                identb_sb[:, :])
                        return _f
                    for cch in range(4):
                        ext.append(vtr(cch))

                    def vcopy():
                        nc.scalar.copy(V_sb[:, tb * 4:(tb + 1) * 4, :],
                                       vt[:, :, :])
                    ext.append(vcopy)
                    return ext

                extras = []
                for tb in range(NTB):
                    drains = emit_proj(tb, extras)
                    extras = extras + make_extras(tb, *drains)
                for e in extras:
                    e()

            def emit_collective(c):
                if no_collective:
                    nc.sync.dma_start(out=a2a_out[c], in_=a2a_in[c])
                else:
                    nc.gpsimd.collective_compute(
                        "AllToAll", ALU.bypass,
                        replica_groups=[list(range(NC_CORES))],
                        ins=[a2a_in[c].opt()], outs=[a2a_out[c].opt()],
                    )

            # ============ Phase 2+3: attention, collectives, wo ============
            with (
                tc.tile_pool(name="battn", bufs=1) as battn,
                tc.tile_pool(name="apt", bufs=6) as aptpool,
                tc.tile_pool(name="aden", bufs=2) as adenpool,
                tc.tile_pool(name="amisc", bufs=4) as amisc,
                tc.tile_pool(name="wao", bufs=1) as waopool,
                tc.tile_pool(name="wwt", bufs=64) as wwtpool,
                tc.tile_pool(name="wdr", bufs=2) as wdrpool,
                tc.tile_pool(name="outps", bufs=3, space="PSUM") as outps,
                tc.tile_pool(name="wops", bufs=1, space="PSUM") as wops,
                tc.tile_pool(name="sps", bufs=2, space="PSUM") as spsps,
            ):
                mask_sb = battn.tile([128, 2, QBS], BF16)
                nc.scalar.dma_start(out=mask_sb, in_=biasd)
                maskh_sb = battn.tile([128, 2, QH], BF16)
                nc.scalar.dma_start(out=maskh_sb, in_=biashd)

                # wo weight tiles, loaded once, fully resident (64KB/part)
                wt_tiles = {}
                for eb in range(DIM // 512):
                    for hc in range(ECH):
                        wt = wwtpool.tile([128, 512], BF16, tag="wt",
                                          name=f"wt{eb}_{hc}")
                        nc.sync.dma_start(
                            out=wt,
                            in_=woT[hc * 128:(hc + 1) * 128,
                                    eb * 512:(eb + 1) * 512])
                        wt_tiles[(eb, hc)] = wt

                ao_sbs = {}

                def load_aosb(c):
                    w = QBS if c == 0 else QH
                    ao_sb = waopool.tile([128, ECH, w], BF16, bufs=1,
                                         tag=f"aosb{c}", name=f"aosb{c}")
                    aor = a2a_out[c].rearrange("r h p t -> p (r h) t")
                    for hc in range(ECH):
                        nc.gpsimd.dma_start(out=ao_sb[:, hc, :],
                                            in_=aor[:, hc, :])
                    ao_sbs[c] = ao_sb

                # wo work: one closure per (chunk, eb, tt) = 16 matmuls;
                # chunk 0 has tt 0..3 (512 q), chunks 1,2 have tt 0..1
                def wo_group(c, eb, tt):
                    def _f():
                        ao_sb = ao_sbs[c]
                        ops = wops.tile([128, 512], F32, tag="wps",
                                        name=f"wps{c}_{eb}_{tt}")
                        for hc in range(ECH):
                            nc.tensor.matmul(
                                ops[:, :],
                                ao_sb[:, hc, tt * 128:(tt + 1) * 128],
                                wt_tiles[(eb, hc)],
                                start=(hc == 0), stop=(hc == ECH - 1),
                                skip_group_check=True)
                        od = wdrpool.tile([128, 512], F32, tag="od")
                        if (eb + tt) % 2 == 0:
                            nc.scalar.copy(od[:, :], ops[:, :])
                        else:
                            nc.vector.tensor_copy(od[:, :], ops[:, :])
                        row = (QBS if c == 0 else (c - 1) * QH) + tt * 128
                        nc.sync.dma_start(
                            out=out[row:row + 128,
                                    eb * 512:(eb + 1) * 512],
                            in_=od)
                    return _f

                wo_pending = []

                # software-pipelined PV: deferred one group behind scores
                pending = []

                def flush_pv():
                    while pending:
                        pt, o_ps, b_, g_, ng_ = pending.pop(0)
                        last = g_ == ng_ - 1
                        qw = QH if last else QBS
                        q0 = QBS - qw
                        for ci in range(2):
                            kcol = b_ * seq + g_ * 256 + ci * 128
                            nc.tensor.matmul(
                                o_ps[:, q0:QBS],
                                V_sb[:, kcol // 128, :],
                                pt[:, ci, 0:qw],
                                start=(g_ == 0 and ci == 0),
                                stop=(last and ci == 1),
                                skip_group_check=True)

                finish = []

                def flush_finish():
                    while finish:
                        finish.pop(0)()

                # odd q-blocks first (their a2a chunk fires mid-attention and
                # is fully hidden by the even phase); within each parity, big
                # and small q-blocks alternate so small units' serial finish
                # chains hide inside big units' PE work.
                def mix_qbs(par):
                    qbs = list(range(QB - 1 - (1 - par), -1, -2))
                    mix = []
                    while qbs:
                        mix.append(qbs.pop(0))       # biggest
                        if qbs:
                            mix.append(qbs.pop())    # smallest
                    return mix

                unit_order = [(b, qb) for par in (1, 0)
                              for qb in mix_qbs(par)
                              for b in range(BS)]
                n_odd = len(unit_order) // 2
                last_ao = [None]
                for ui, (b, qb) in enumerate(unit_order):
                    tok0 = b * seq + qb * QBS
                    ng = 2 * (qb + 1)
                    for h in range(HPC):
                        o_ps = outps.tile([128, QBS], F32, tag="ops")
                        den_sb = adenpool.tile([128, QBS], BF16, tag="den")
                        for g in range(ng):
                            last = g == ng - 1
                            qw = QH if last else QBS
                            q0 = QBS - qw
                            sps = spsps.tile([128, 2, QBS], F32, tag="sps")
                            for ci in range(2):
                                kcol = b * seq + g * 256 + ci * 128
                                nc.tensor.matmul(
                                    sps[:, ci, 0:qw],
                                    K_sb[:, kcol:kcol + 128],
                                    Q_sb[:, h, tok0 + q0:tok0 + QBS],
                                    start=True, stop=True,
                                    skip_group_check=True)
                            flush_pv()
                            flush_finish()
                            pt = aptpool.tile([128, 2, QBS], BF16, tag="pt")
                            nc.scalar.activation(pt[:, :, 0:qw],
                                                 sps[:, :, 0:qw], AF.Exp)
                            if g == ng - 2:
                                nc.vector.tensor_mul(pt[:, :, :],
                                                     pt[:, :, :],
                                                     mask_sb[:, :, :])
                            elif last:
                                nc.vector.tensor_mul(pt[:, :, 0:qw],
                                                     pt[:, :, 0:qw],
                                                     maskh_sb[:, :, :])
                            pending.append((pt, o_ps, b, g, ng))
                            if g == 0:
                                nc.vector.tensor_add(den_sb[:, :],
                                                     pt[:, 0, :],
                                                     pt[:, 1, :])
                            else:
                                tmp = amisc.tile([128, QBS], BF16,
                                                 tag="dtmp")
                                nc.vector.tensor_add(
                                    tmp[:, 0:qw],
                                    pt[:, 0, 0:qw],
                                    pt[:, 1, 0:qw])
                                nc.vector.tensor_add(
                                    den_sb[:, q0:QBS],
                                    tmp[:, 0:qw],
                                    den_sb[:, q0:QBS])

                        def unit_finish(o_ps=o_ps, den_sb=den_sb,
                                        tok0=tok0, h=h, qb=qb):
                            den_ps = outps.tile([128, QBS], F32, tag="ops",
                                                name=f"dps{tok0}_{h}")
                            nc.tensor.matmul(den_ps[:, :], onesb_sb[:, :],
                                             den_sb[:, :], start=True,
                                             stop=True, skip_group_check=True)
                            rv = amisc.tile([128, QBS], F32, tag="arv")
                            nc.vector.reciprocal(rv[:, :], den_ps[:, :])
                            ao = amisc.tile([128, QBS], BF16, tag="ao")
                            nc.vector.tensor_mul(ao[:, :], o_ps[:, :],
                                                 rv[:, :])
                            last_ao[0] = ao
                            d = tok0 // TPC
                            if qb % 2 == 1:
                                nc.sync.dma_start(
                                    out=a2a_in[0][d, h, :, :], in_=ao[:, :])
                            else:
                                for j in (1, 2):
                                    nc.sync.dma_start(
                                        out=a2a_in[j][d, h, :, :],
                                        in_=ao[:, (j - 1) * QH:j * QH])

                        finish.append(unit_finish)
                    if ui == n_odd - 1:
                        flush_pv()
                        flush_finish()
                        emit_collective(0)
                    if ui == n_odd + wo_start:
                        # Pre-write garbage (a real data dep on the current
                        # attention unit) into one column of every (hc, tt)
                        # region of ao_sb0 halves, THEN emit the loads that
                        # overwrite it. The tile scheduler models collectives
                        # as ~instant; without this extra writer it would
                        # place chunk-0 wo matmuls at a stream position
                        # reached long before the collective lands at
                        # runtime, freezing the in-order PE pipe. With it,
                        # the scheduler's readiness estimate for wo (~this
                        # unit's finish) matches runtime (collective landed).
                        # The q-half tt 2,3 is gated on the LAST even unit so
                        # its 8 wo groups remain to cover the collective-1
                        # window in the tail.
                        ao_sb = waopool.tile([128, ECH, QBS], BF16, bufs=1,
                                             tag="aosb0", name="aosb0")
                        ao_sbs[0] = ao_sb
                        for tt in range(4):
                            nc.vector.tensor_copy(
                                ao_sb[:, :, tt * 128],
                                last_ao[0][:, 0:ECH])
                        aor = a2a_out[0].rearrange("r h p t -> p (r h) t")
                        for hc in range(ECH):
                            nc.gpsimd.dma_start(out=ao_sb[:, hc, :],
                                                in_=aor[:, hc, :])
                flush_pv()
                flush_finish()
                emit_collective(1)
                emit_collective(2)
                load_aosb(1)
                load_aosb(2)
                # wo for the odd chunk runs while collectives 1+2 transfer
                for eb in range(DIM // 512):
                    for tt in range(4):
                        wo_group(0, eb, tt)()
                for eb in range(DIM // 512):
                    for tt in range(2):
                        wo_group(1, eb, tt)()
                for eb in range(DIM // 512):
                    for tt in range(2):
                        wo_group(2, eb, tt)()
    nc.compile()
    return nc


# ---------------- host-side prep / run ----------------

_PROG_CACHE = {}


def _get_program(seq):
    if seq not in _PROG_CACHE:
        _PROG_CACHE[seq] = build_program(seq)
    return _PROG_CACHE[seq]


def _rot_perm():
    return np.concatenate([np.arange(0, HD, 2), np.arange(1, HD, 2)])


def make_inputs(x, freqs_cis, wq, wk, wv, wo, q_norm_w, k_norm_w):
    bs, seq, _ = x.shape
    T = bs * seq
    perm = _rot_perm()

    xT = np.ascontiguousarray(
        x.reshape(T, DIM).T.astype(ml_dtypes.bfloat16))
    woT = np.ascontiguousarray(wo.T.astype(ml_dtypes.bfloat16))
    cos = freqs_cis[:, :, 0].T.astype(np.float32)   # [64, seq]
    sin = freqs_cis[:, :, 1].T.astype(np.float32)
    ccd = np.ascontiguousarray(
        np.concatenate([cos, cos], axis=0).astype(ml_dtypes.bfloat16))
    ssd = np.ascontiguousarray(
        np.concatenate([-sin, sin], axis=0).astype(ml_dtypes.bfloat16))

    # additive causal-mask bias tiles (0 allowed / NEG masked)
    # full tile: second-to-last 256-key group vs all 512 q
    # half tile: last 256-key group vs q in [256, 512) only
    masks = np.zeros((2, 128, 2, QBS), dtype=np.float32)
    for s in range(2):
        for c in range(2):
            k_rel = s * 256 + c * 128 + np.arange(128)[:, None]
            masks[s, :, c, :] = (k_rel <= np.arange(QBS)[None, :])
    # multiplicative masks: unmasked -> 1, masked -> 0
    biasd = np.ascontiguousarray(masks[0].astype(ml_dtypes.bfloat16))
    biashd = np.ascontiguousarray(
        masks[1, :, :, QH:].astype(ml_dtypes.bfloat16))

    onesd = np.ones((128, 128), dtype=np.float32)
    identbd = np.eye(128, dtype=ml_dtypes.bfloat16)

    in_maps = []
    for c in range(NC_CORES):
        g = c // 2
        wq_rows = wq[c * HPC * HD:(c + 1) * HPC * HD].reshape(HPC, HD, DIM)
        wq_rows = wq_rows[:, perm, :].reshape(HPC * HD, DIM)
        wk_rows = wk[g * HD:(g + 1) * HD][perm]
        wv_rows = wv[g * HD:(g + 1) * HD]
        in_maps.append({
            "xT": xT,
            "wqT": np.ascontiguousarray(
                wq_rows.T.astype(ml_dtypes.bfloat16)),
            "wkT": np.ascontiguousarray(
                wk_rows.T.astype(ml_dtypes.bfloat16)),
            "wvT": np.ascontiguousarray(
                wv_rows.T.astype(ml_dtypes.bfloat16)),
            "woT": woT,
            "ccd": ccd,
            "ssd": ssd,
            "biasd": biasd,
            "biashd": biashd,
            "onesd": onesd,
            "identbd": identbd,
        })
    return in_maps


def run(x, freqs_cis, wq, wk, wv, wo, q_norm_w, k_norm_w, trace=False):
    bs, seq, _ = x.shape
    nc = _get_program(seq)
    in_maps = make_inputs(x, freqs_cis, wq, wk, wv, wo, q_norm_w, k_norm_w)
    res = None
    for attempt in range(3):
        try:
            res = run_bass_kernel_spmd(nc, in_maps, list(range(NC_CORES)),
                                       trace=trace)
            break
        except Exception:
            if attempt == 2:
                raise
    shards = [res.results[c]["out"] for c in range(NC_CORES)]
    full = np.concatenate(shards, axis=0).reshape(bs, seq, DIM)
    return full, res


def kernel(x, freqs_cis, wq, wk, wv, wo, q_norm_w, k_norm_w):
    out, _ = run(np.asarray(x, np.float32), np.asarray(freqs_cis, np.float32),
                 np.asarray(wq, np.float32), np.asarray(wk, np.float32),
                 np.asarray(wv, np.float32), np.asarray(wo, np.float32),
                 np.asarray(q_norm_w, np.float32), np.asarray(k_norm_w, np.float32))
    return out
